# revision 1
# baseline (speedup 1.0000x reference)
"""Grouped SwiGLU FFN (8 experts) — expert-parallel Bass kernel for 8 trn2 cores.

Per core (one expert): out = (silu(x@w1) * (x@w3T)) @ w2T, all fp32.
  x: [T=1024, D=2048], w1: [D, H=4096], w3: [H, D], w2: [D, H].

Device-side formulation (all matmuls in float32r at full PE rate, zero
on-device transposes — layouts are pre-packed on host):
  phase1: g^T[h, t]  = silu(w1^T-tile.T @ x^T) * (w3-tile.T @ x^T)   (per h-tile)
  phase2: out^T[d,t] = sum_h w2-tile.T @ g^T                          (w2 stationary)
H is processed in 8 slices of 512 (4 h-tiles); out^T accumulated in SBUF fp32.
Host unpacks outT -> out.
"""

import sys

sys.path.insert(0, "/opt/trn_rl_repo")

import numpy as np

import concourse.bass as bass
from concourse import bacc
import concourse.mybir as mybir
import concourse.tile as tile
from concourse.bass_utils import run_bass_kernel_spmd

E, T, D, H = 8, 1024, 2048, 4096
P = 128
NT = 512            # matmul moving free dim (fp32 max)
DT = D // P         # 16 contraction tiles over D
HT = H // P         # 32 h-tiles
HQ = 4              # h-phases
HTQ = HT // HQ      # 4 h-tiles per phase
TH = T // NT        # 2 t-halves
DTT = D // P        # 16 out^T row tiles
F32 = mybir.dt.float32
F32R = mybir.dt.float32r

_CACHE: dict = {}
USE_SILU = True


def _build_nc():
    nc = bacc.Bacc("TRN2", target_bir_lowering=False, debug=False)
    xp = nc.dram_tensor("xp", [DT, P, T], F32R, kind="ExternalInput")
    w1p = nc.dram_tensor("w1p", [HT, P, DT, P], F32R, kind="ExternalInput")
    w3p = nc.dram_tensor("w3p", [HT, P, DT, P], F32R, kind="ExternalInput")
    w2p = nc.dram_tensor("w2p", [HQ, DTT, P, HTQ, P], F32R, kind="ExternalInput")
    outT = nc.dram_tensor("outT", [D, T], F32, kind="ExternalOutput")

    with tile.TileContext(nc) as tc:
        with (
            tc.tile_pool(name="xpool", bufs=1) as xpool,
            tc.tile_pool(name="gpool", bufs=1) as gpool,
            tc.tile_pool(name="opool", bufs=1) as opool,
            tc.tile_pool(name="wpool", bufs=2) as wpool,
            tc.tile_pool(name="w2pool", bufs=3) as w2pool,
            tc.tile_pool(name="spool", bufs=1) as spool,
            tc.tile_pool(name="pspool", bufs=8, space="PSUM") as pspool,
        ):
            def load_w(ht):
                w1sb = wpool.tile([P, DT, P], F32R, tag="w1", name=f"w1sb_{ht}")
                nc.sync.dma_start(w1sb, w1p[ht])
                w3sb = wpool.tile([P, DT, P], F32R, tag="w3", name=f"w3sb_{ht}")
                nc.sync.dma_start(w3sb, w3p[ht])
                return w1sb, w3sb

            # first weight tiles before the bulk x load so PE starts ASAP
            w_pre = load_w(0)
            xsb = xpool.tile([P, DT, T], F32R, tag="x")
            # th=0 halves first: lets early psum groups complete while the
            # second half of x is still in flight
            for th in range(TH):
                for dt_i in range(DT):
                    sl = slice(th * NT, (th + 1) * NT)
                    nc.sync.dma_start(xsb[:, dt_i, sl], xp[dt_i, :, sl])
            out_acc = opool.tile([P, DTT, T], F32, tag="oacc")

            def mm_burst(ps, wsb, th, dts):
                ts = slice(th * NT, (th + 1) * NT)
                for dt_i in dts:
                    nc.tensor.matmul(
                        ps,
                        lhsT=wsb[:, dt_i],
                        rhs=xsb[:, dt_i, ts],
                        start=(dt_i == 0),
                        stop=(dt_i == DT - 1),
                    )

            def epilogue(ps1, ps3, g, htl, th):
                ts = slice(th * NT, (th + 1) * NT)
                sil = spool.tile([P, NT], F32, tag="sil")
                if USE_SILU:
                    nc.scalar.activation(
                        sil, ps1, mybir.ActivationFunctionType.Silu
                    )
                else:
                    sg = spool.tile([P, NT], F32, tag="sg")
                    nc.scalar.activation(
                        sg, ps1, mybir.ActivationFunctionType.Sigmoid
                    )
                    nc.vector.tensor_mul(out=sil, in0=sg, in1=ps1)
                nc.vector.tensor_mul(out=g[:, htl, ts], in0=sil, in1=ps3)

            for hq in range(HQ):
                g = gpool.tile([P, HTQ, T], F32R, tag="g")
                if hq == 0:
                    # warm start: split the first two h-tiles' accumulation
                    # into dt halves so 8 psum groups are in the PE queue
                    # while the x chunks stream in
                    w_next = load_w(1)
                    warm = {}
                    for htl in range(2):
                        wsb = w_pre if htl == 0 else w_next
                        for th in range(TH):
                            ps1 = pspool.tile([P, NT], F32, tag="ps", bufs=4, name="ps1")
                            ps3 = pspool.tile([P, NT], F32, tag="ps", bufs=4, name="ps3")
                            warm[htl, th] = (ps1, ps3, wsb)
                            mm_burst(ps1, wsb[0], th, range(DT // 2))
                            mm_burst(ps3, wsb[1], th, range(DT // 2))
                    for htl in range(2):
                        for th in range(TH):
                            ps1, ps3, wsb = warm[htl, th]
                            mm_burst(ps1, wsb[0], th, range(DT // 2, DT))
                            mm_burst(ps3, wsb[1], th, range(DT // 2, DT))
                            epilogue(ps1, ps3, g, htl, th)
                    first_htl = 2
                else:
                    first_htl = 0
                for htl in range(first_htl, HTQ):
                    ht = hq * HTQ + htl
                    w1sb, w3sb = w_pre if htl == 0 else load_w(ht)
                    for th in range(TH):
                        ps1 = pspool.tile([P, NT], F32, tag="ps", bufs=4, name="ps1")
                        ps3 = pspool.tile([P, NT], F32, tag="ps", bufs=4, name="ps3")
                        mm_burst(ps1, w1sb, th, range(DT))
                        mm_burst(ps3, w3sb, th, range(DT))
                        epilogue(ps1, ps3, g, htl, th)

                # prefetch next phase's first weight tiles ahead of the w2
                # stream so phase1 of hq+1 starts without a DMA bubble
                if hq + 1 < HQ:
                    w_pre = load_w((hq + 1) * HTQ)

                for dtt in range(DTT):
                    w2sb = w2pool.tile([P, HTQ, P], F32R, tag="w2")
                    nc.sync.dma_start(w2sb, w2p[hq, dtt])
                    # one double-bank psum for both t-halves: half the
                    # accumulation groups and DVE ops in phase 2
                    po = pspool.tile([P, T], F32, tag="po2", bufs=2, name="po")
                    for th in range(TH):
                        ts = slice(th * NT, (th + 1) * NT)
                        for htl in range(HTQ):
                            nc.tensor.matmul(
                                po[:, ts],
                                lhsT=w2sb[:, htl],
                                rhs=g[:, htl, ts],
                                start=(htl == 0),
                                stop=(htl == HTQ - 1),
                            )
                    if hq == 0:
                        nc.vector.tensor_copy(out=out_acc[:, dtt], in_=po)
                    else:
                        nc.vector.tensor_add(
                            out=out_acc[:, dtt],
                            in0=out_acc[:, dtt],
                            in1=po,
                        )

            for dtt in range(DTT):
                for th in range(TH):
                    sl = slice(th * NT, (th + 1) * NT)
                    nc.sync.dma_start(
                        outT[dtt * P : (dtt + 1) * P, sl], out_acc[:, dtt, sl]
                    )
    nc.compile()
    return nc


def _round_fp32r(a):
    """Round fp32 to the fp32r grid: 11 explicit mantissa bits (low 12 bits
    zero), round-to-nearest-even — what the PE consumes at full rate."""
    u = np.ascontiguousarray(a, dtype=np.float32).view(np.uint32)
    low = u & np.uint32(0xFFF)
    base = u & np.uint32(0xFFFFF000)
    lsb = (base >> np.uint32(12)) & np.uint32(1)
    roundup = (low > 0x800) | ((low == 0x800) & (lsb == 1))
    out = base + (roundup.astype(np.uint32) << np.uint32(12))
    return out.view(np.float32)


def _pack_inputs(x, w1, w2, w3):
    """Per-expert host-side packing into DMA-linear layouts."""
    in_maps = []
    for e in range(E):
        xe = _round_fp32r(np.asarray(x[e], dtype=np.float32))
        w1e = _round_fp32r(np.asarray(w1[e], dtype=np.float32))
        w2e = _round_fp32r(np.asarray(w2[e], dtype=np.float32))
        w3e = _round_fp32r(np.asarray(w3[e], dtype=np.float32))
        # xp[dt, p, t] = x[t, dt*128+p]
        xp = np.ascontiguousarray(xe.reshape(T, DT, P).transpose(1, 2, 0))
        # w1p[ht, p, dt, h] = w1[dt*128+p, ht*128+h]
        w1p = np.ascontiguousarray(
            w1e.reshape(DT, P, HT, P).transpose(2, 1, 0, 3)
        )
        # w3p[ht, p, dt, h] = w3[ht*128+h, dt*128+p]
        w3p = np.ascontiguousarray(
            w3e.reshape(HT, P, DT, P).transpose(0, 3, 2, 1)
        )
        # w2p[hq, dtt, p, htl, d] = w2[dtt*128+d, (hq*HTQ+htl)*128+p]
        w2p = np.ascontiguousarray(
            w2e.reshape(DTT, P, HQ, HTQ, P).transpose(2, 0, 4, 3, 1)
        )
        in_maps.append({"xp": xp, "w1p": w1p, "w3p": w3p, "w2p": w2p})
    return in_maps


def kernel(x, w1, w2, w3, _trace=False, _trace_kwargs=None):
    if "nc" not in _CACHE:
        _CACHE["nc"] = _build_nc()
    nc = _CACHE["nc"]
    in_maps = _pack_inputs(x, w1, w2, w3)
    kw = {}
    if _trace:
        kw = {"trace": True}
        if _trace_kwargs:
            kw.update(_trace_kwargs)
    res = run_bass_kernel_spmd(nc, in_maps, core_ids=list(range(E)), **kw)
    out = np.empty((E, T, D), dtype=np.float32)
    for e in range(E):
        out[e] = res.results[e]["outT"].T
    if _trace:
        _CACHE["last_results"] = res
    return out



# revision 2
# speedup vs baseline: 1.0675x; 1.0675x over previous
"""Grouped SwiGLU FFN (8 experts) — expert-parallel Bass kernel for 8 trn2 cores.

Per core (one expert): out = (silu(x@w1) * (x@w3T)) @ w2T.
  x: [T=1024, D=2048], w1: [D, H=4096], w3: [H, D], w2: [D, H].

Device-side formulation (matmul operands in fp16 at full PE rate — halves
LDWEIGHTS time vs fp32r so the PE pitch hits the 1-col/cycle streaming
bound — zero on-device transposes; layouts are pre-packed on host):
  phase1: g^T[h, t]  = silu(w1^T-tile.T @ x^T) * (w3-tile.T @ x^T)   (per h-tile)
  phase2: out^T[d,t] = sum_h w2-tile.T @ g^T                          (w2 stationary)
H is processed in 4 phases of 1024 (8 h-tiles); out^T accumulated in SBUF fp32.
Host unpacks outT -> out.
"""

import sys

sys.path.insert(0, "/opt/trn_rl_repo")

import numpy as np

import concourse.bass as bass
from concourse import bacc
import concourse.mybir as mybir
import concourse.tile as tile
from concourse.bass_utils import run_bass_kernel_spmd

E, T, D, H = 8, 1024, 2048, 4096
P = 128
NT = 512            # matmul moving free dim (psum bank limit for fp32 out)
DT = D // P         # 16 contraction tiles over D
HT = H // P         # 32 h-tiles
HQ = 4              # h-phases
HTQ = HT // HQ      # 8 h-tiles per phase
TH = T // NT        # 2 t-halves
DTT = D // P        # 16 out^T row tiles
HD = DT // 2        # half of the contraction tiles (warm-start split)
F32 = mybir.dt.float32
F16 = mybir.dt.float16

_CACHE: dict = {}


def _build_nc():
    nc = bacc.Bacc("TRN2", target_bir_lowering=False, debug=False)
    xp = nc.dram_tensor("xp", [DT, P, T], F16, kind="ExternalInput")
    w1p = nc.dram_tensor("w1p", [HT, P, DT, P], F16, kind="ExternalInput")
    w3p = nc.dram_tensor("w3p", [HT, P, DT, P], F16, kind="ExternalInput")
    w2p = nc.dram_tensor("w2p", [HQ, DTT, P, HTQ, P], F16, kind="ExternalInput")
    outT = nc.dram_tensor("outT", [D, T], F32, kind="ExternalOutput")

    with tile.TileContext(nc) as tc:
        with (
            tc.tile_pool(name="xpool", bufs=1) as xpool,
            tc.tile_pool(name="gpool", bufs=1) as gpool,
            tc.tile_pool(name="opool", bufs=1) as opool,
            tc.tile_pool(name="wpool", bufs=2) as wpool,
            tc.tile_pool(name="w2pool", bufs=3) as w2pool,
            tc.tile_pool(name="spool", bufs=1) as spool,
            tc.tile_pool(name="pspool", bufs=8, space="PSUM") as pspool,
        ):
            def load_w(ht):
                w1sb = wpool.tile([P, DT, P], F16, tag="w1", name=f"w1sb_{ht}")
                nc.sync.dma_start(w1sb, w1p[ht])
                w3sb = wpool.tile([P, DT, P], F16, tag="w3", name=f"w3sb_{ht}")
                nc.sync.dma_start(w3sb, w3p[ht])
                return w1sb, w3sb

            # --- startup: interleave DMA issue in exact first-use order so
            # the PE's warm bursts start as soon as possible
            w1sb0 = wpool.tile([P, DT, P], F16, tag="w1", name="w1sb_0")
            w3sb0 = wpool.tile([P, DT, P], F16, tag="w3", name="w3sb_0")
            w1sb1 = wpool.tile([P, DT, P], F16, tag="w1", name="w1sb_1")
            w3sb1 = wpool.tile([P, DT, P], F16, tag="w3", name="w3sb_1")
            xsb = xpool.tile([P, DT, T], F16, tag="x")

            def wdma(sb, src, half):
                sl = slice(half * HD, (half + 1) * HD)
                nc.sync.dma_start(sb[:, sl], src[:, sl])

            def xdma(th, half):
                ts = slice(th * NT, (th + 1) * NT)
                for dt_i in range(half * HD, (half + 1) * HD):
                    nc.sync.dma_start(xsb[:, dt_i, ts], xp[dt_i, :, ts])

            # dependency order of the 8 warm bursts + 4 completions below
            wdma(w1sb0, w1p[0], 0)
            xdma(0, 0)
            wdma(w3sb0, w3p[0], 0)
            xdma(1, 0)
            wdma(w1sb1, w1p[1], 0)
            wdma(w3sb1, w3p[1], 0)
            wdma(w1sb0, w1p[0], 1)
            wdma(w3sb0, w3p[0], 1)
            xdma(0, 1)
            xdma(1, 1)
            wdma(w1sb1, w1p[1], 1)
            wdma(w3sb1, w3p[1], 1)

            w_pre = (w1sb0, w3sb0)
            w_next = (w1sb1, w3sb1)
            out_acc = opool.tile([P, DTT, T], F32, tag="oacc")

            def mm_burst(ps, wsb, th, dts):
                ts = slice(th * NT, (th + 1) * NT)
                for dt_i in dts:
                    nc.tensor.matmul(
                        ps,
                        lhsT=wsb[:, dt_i],
                        rhs=xsb[:, dt_i, ts],
                        start=(dt_i == 0),
                        stop=(dt_i == DT - 1),
                    )

            def epilogue(ps1, ps3, g, htl, th):
                ts = slice(th * NT, (th + 1) * NT)
                sil = spool.tile([P, NT], F32, tag="sil")
                nc.scalar.activation(
                    sil, ps1, mybir.ActivationFunctionType.Silu
                )
                nc.vector.tensor_mul(out=g[:, htl, ts], in0=sil, in1=ps3)

            for hq in range(HQ):
                g = gpool.tile([P, HTQ, T], F16, tag="g")
                if hq == 0:
                    # warm start: split the first two h-tiles' accumulation
                    # into dt halves so 8 psum groups are in the PE queue
                    # while the x chunks stream in
                    warm = {}
                    for htl in range(2):
                        wsb = w_pre if htl == 0 else w_next
                        for th in range(TH):
                            ps1 = pspool.tile([P, NT], F32, tag="ps", bufs=4, name="ps1")
                            ps3 = pspool.tile([P, NT], F32, tag="ps", bufs=4, name="ps3")
                            warm[htl, th] = (ps1, ps3, wsb)
                            mm_burst(ps1, wsb[0], th, range(HD))
                            mm_burst(ps3, wsb[1], th, range(HD))
                    for htl in range(2):
                        for th in range(TH):
                            ps1, ps3, wsb = warm[htl, th]
                            mm_burst(ps1, wsb[0], th, range(HD, DT))
                            mm_burst(ps3, wsb[1], th, range(HD, DT))
                            epilogue(ps1, ps3, g, htl, th)
                    first_htl = 2
                else:
                    first_htl = 0
                for htl in range(first_htl, HTQ):
                    ht = hq * HTQ + htl
                    w1sb, w3sb = w_pre if htl == 0 else load_w(ht)
                    for th in range(TH):
                        ps1 = pspool.tile([P, NT], F32, tag="ps", bufs=4, name="ps1")
                        ps3 = pspool.tile([P, NT], F32, tag="ps", bufs=4, name="ps3")
                        mm_burst(ps1, w1sb, th, range(DT))
                        mm_burst(ps3, w3sb, th, range(DT))
                        epilogue(ps1, ps3, g, htl, th)

                # prefetch next phase's first weight tiles ahead of the w2
                # stream so phase1 of hq+1 starts without a DMA bubble
                if hq + 1 < HQ:
                    w_pre = load_w((hq + 1) * HTQ)

                for dtt in range(DTT):
                    w2sb = w2pool.tile([P, HTQ, P], F16, tag="w2")
                    nc.sync.dma_start(w2sb, w2p[hq, dtt])
                    # one double-bank psum for both t-halves: half the
                    # accumulation groups and DVE ops in phase 2
                    po = pspool.tile([P, T], F32, tag="po2", bufs=2, name="po")
                    for th in range(TH):
                        ts = slice(th * NT, (th + 1) * NT)
                        for htl in range(HTQ):
                            nc.tensor.matmul(
                                po[:, ts],
                                lhsT=w2sb[:, htl],
                                rhs=g[:, htl, ts],
                                start=(htl == 0),
                                stop=(htl == HTQ - 1),
                            )
                    if hq == 0:
                        nc.vector.tensor_copy(out=out_acc[:, dtt], in_=po)
                    else:
                        nc.vector.tensor_add(
                            out=out_acc[:, dtt],
                            in0=out_acc[:, dtt],
                            in1=po,
                        )
                    if hq == HQ - 1:
                        # final value for this dtt: stream it out now so the
                        # output DMA fully overlaps the remaining compute
                        for th in range(TH):
                            ts = slice(th * NT, (th + 1) * NT)
                            nc.sync.dma_start(
                                outT[dtt * P : (dtt + 1) * P, ts],
                                out_acc[:, dtt, ts],
                            )
    nc.compile()
    return nc


def _pack_inputs(x, w1, w2, w3):
    """Per-expert host-side packing into DMA-linear fp16 layouts."""
    in_maps = []
    for e in range(E):
        xe = np.asarray(x[e], dtype=np.float16)
        w1e = np.asarray(w1[e], dtype=np.float16)
        w2e = np.asarray(w2[e], dtype=np.float16)
        w3e = np.asarray(w3[e], dtype=np.float16)
        # xp[dt, p, t] = x[t, dt*128+p]
        xp = np.ascontiguousarray(xe.reshape(T, DT, P).transpose(1, 2, 0))
        # w1p[ht, p, dt, h] = w1[dt*128+p, ht*128+h]
        w1p = np.ascontiguousarray(
            w1e.reshape(DT, P, HT, P).transpose(2, 1, 0, 3)
        )
        # w3p[ht, p, dt, h] = w3[ht*128+h, dt*128+p]
        w3p = np.ascontiguousarray(
            w3e.reshape(HT, P, DT, P).transpose(0, 3, 2, 1)
        )
        # w2p[hq, dtt, p, htl, d] = w2[dtt*128+d, (hq*HTQ+htl)*128+p]
        w2p = np.ascontiguousarray(
            w2e.reshape(DTT, P, HQ, HTQ, P).transpose(2, 0, 4, 3, 1)
        )
        in_maps.append({"xp": xp, "w1p": w1p, "w3p": w3p, "w2p": w2p})
    return in_maps


def kernel(x, w1, w2, w3, _trace=False, _trace_kwargs=None):
    if "nc" not in _CACHE:
        _CACHE["nc"] = _build_nc()
    nc = _CACHE["nc"]
    in_maps = _pack_inputs(x, w1, w2, w3)
    kw = {}
    if _trace:
        kw = {"trace": True}
        if _trace_kwargs:
            kw.update(_trace_kwargs)
    res = run_bass_kernel_spmd(nc, in_maps, core_ids=list(range(E)), **kw)
    out = np.empty((E, T, D), dtype=np.float32)
    for e in range(E):
        out[e] = res.results[e]["outT"].T
    if _trace:
        _CACHE["last_results"] = res
    return out


# revision 3
# speedup vs baseline: 1.0707x; 1.0030x over previous
"""Grouped SwiGLU FFN (8 experts) — expert-parallel Bass kernel for 8 trn2 cores.

Per core (one expert): out = (silu(x@w1) * (x@w3T)) @ w2T.
  x: [T=1024, D=2048], w1: [D, H=4096], w3: [H, D], w2: [D, H].

Device-side formulation (matmul operands in fp16 at full PE rate — halves
LDWEIGHTS time vs fp32r so the PE pitch hits the 1-col/cycle streaming
bound — zero on-device transposes; layouts are pre-packed on host):
  phase1: g^T[h, t]  = silu(w1^T-tile.T @ x^T) * (w3-tile.T @ x^T)   (per h-tile)
  phase2: out^T[d,t] = sum_h w2-tile.T @ g^T                          (w2 stationary)
H is processed in 4 phases of 1024 (8 h-tiles); out^T accumulated in SBUF fp32.
Host unpacks outT -> out.

Startup: dummy "burn" matmuls on scratch SBUF ramp the PE clock while the
DMA rings initialize; startup DMAs are issued in exact first-use order.
"""

import sys

sys.path.insert(0, "/opt/trn_rl_repo")

import numpy as np

import concourse.bass as bass
from concourse import bacc
import concourse.mybir as mybir
import concourse.tile as tile
from concourse.bass_utils import run_bass_kernel_spmd

E, T, D, H = 8, 1024, 2048, 4096
P = 128
NT = 512            # matmul moving free dim (psum bank limit for fp32 out)
DT = D // P         # 16 contraction tiles over D
HT = H // P         # 32 h-tiles
HQ = 4              # h-phases
HTQ = HT // HQ      # 8 h-tiles per phase
TH = T // NT        # 2 t-halves
DTT = D // P        # 16 out^T row tiles
HD = DT // 2        # half of the contraction tiles (warm-start split)
NBURN = 40          # clock-ramp matmuls issued before real work
F32 = mybir.dt.float32
F16 = mybir.dt.float16

_CACHE: dict = {}


def _build_nc():
    nc = bacc.Bacc("TRN2", target_bir_lowering=False, debug=False)
    xp = nc.dram_tensor("xp", [DT, P, T], F16, kind="ExternalInput")
    w1p = nc.dram_tensor("w1p", [HT, P, DT, P], F16, kind="ExternalInput")
    w3p = nc.dram_tensor("w3p", [HT, P, DT, P], F16, kind="ExternalInput")
    w2p = nc.dram_tensor("w2p", [HQ, DTT, P, HTQ, P], F16, kind="ExternalInput")
    outT = nc.dram_tensor("outT", [D, T], F32, kind="ExternalOutput")

    with tile.TileContext(nc) as tc:
        with (
            tc.tile_pool(name="xpool", bufs=1) as xpool,
            tc.tile_pool(name="gpool", bufs=1) as gpool,
            tc.tile_pool(name="opool", bufs=1) as opool,
            tc.tile_pool(name="wpool", bufs=3) as wpool,
            tc.tile_pool(name="w2pool", bufs=3) as w2pool,
            tc.tile_pool(name="spool", bufs=1) as spool,
            tc.tile_pool(name="pspool", bufs=8, space="PSUM") as pspool,
        ):
            # --- PE clock pre-burn on zeroed scratch (no DMA dependency):
            # keeps the Tensor engine busy through the DMA-ring init window
            # so real matmuls start at the full 2.4 GHz p-state
            burnw = spool.tile([P, P], F16, tag="burnw")
            burnx = spool.tile([P, NT], F16, tag="burnx")
            nc.vector.memset(burnw, 0.0)
            nc.vector.memset(burnx, 0.0)
            psb = pspool.tile([P, NT], F32, tag="ps", bufs=4, name="psburn")
            for i in range(NBURN):
                nc.tensor.matmul(
                    psb,
                    lhsT=burnw,
                    rhs=burnx,
                    start=(i == 0),
                    stop=(i == NBURN - 1),
                )

            def load_w(ht):
                w1sb = wpool.tile([P, DT, P], F16, tag="w1", name=f"w1sb_{ht}")
                nc.sync.dma_start(w1sb, w1p[ht])
                w3sb = wpool.tile([P, DT, P], F16, tag="w3", name=f"w3sb_{ht}")
                nc.sync.dma_start(w3sb, w3p[ht])
                return w1sb, w3sb

            # --- startup: interleave DMA issue in exact first-use order so
            # the PE's warm bursts start as soon as possible
            w1sb0 = wpool.tile([P, DT, P], F16, tag="w1", name="w1sb_0")
            w3sb0 = wpool.tile([P, DT, P], F16, tag="w3", name="w3sb_0")
            w1sb1 = wpool.tile([P, DT, P], F16, tag="w1", name="w1sb_1")
            w3sb1 = wpool.tile([P, DT, P], F16, tag="w3", name="w3sb_1")
            xsb = xpool.tile([P, DT, T], F16, tag="x")

            def wdma(sb, src, half):
                sl = slice(half * HD, (half + 1) * HD)
                nc.sync.dma_start(sb[:, sl], src[:, sl])

            def xdma(dt_i):
                # full-T chunk: 2KB contiguous per partition line
                nc.sync.dma_start(xsb[:, dt_i], xp[dt_i])

            wdma(w1sb0, w1p[0], 0)
            for dt_i in range(HD):
                xdma(dt_i)
            wdma(w3sb0, w3p[0], 0)
            wdma(w1sb1, w1p[1], 0)
            wdma(w3sb1, w3p[1], 0)
            wdma(w1sb0, w1p[0], 1)
            wdma(w3sb0, w3p[0], 1)
            for dt_i in range(HD, DT):
                xdma(dt_i)
            wdma(w1sb1, w1p[1], 1)
            wdma(w3sb1, w3p[1], 1)

            w_pre = (w1sb0, w3sb0)
            w_next = (w1sb1, w3sb1)
            out_acc = opool.tile([P, DTT, T], F32, tag="oacc")

            def mm_burst(ps, wsb, th, dts):
                ts = slice(th * NT, (th + 1) * NT)
                for dt_i in dts:
                    nc.tensor.matmul(
                        ps,
                        lhsT=wsb[:, dt_i],
                        rhs=xsb[:, dt_i, ts],
                        start=(dt_i == 0),
                        stop=(dt_i == DT - 1),
                    )

            def epilogue(ps1, ps3, g, htl, th):
                ts = slice(th * NT, (th + 1) * NT)
                sil = spool.tile([P, NT], F32, tag="sil")
                nc.scalar.activation(
                    sil, ps1, mybir.ActivationFunctionType.Silu
                )
                nc.vector.tensor_mul(out=g[:, htl, ts], in0=sil, in1=ps3)

            for hq in range(HQ):
                g = gpool.tile([P, HTQ, T], F16, tag="g")
                if hq == 0:
                    # warm start: split the first two h-tiles' accumulation
                    # into dt halves so 8 psum groups are in the PE queue
                    # while the x chunks stream in
                    warm = {}
                    for htl in range(2):
                        wsb = w_pre if htl == 0 else w_next
                        for th in range(TH):
                            ps1 = pspool.tile([P, NT], F32, tag="ps", bufs=4, name="ps1")
                            ps3 = pspool.tile([P, NT], F32, tag="ps", bufs=4, name="ps3")
                            warm[htl, th] = (ps1, ps3, wsb)
                            mm_burst(ps1, wsb[0], th, range(HD))
                            mm_burst(ps3, wsb[1], th, range(HD))
                    for htl in range(2):
                        for th in range(TH):
                            ps1, ps3, wsb = warm[htl, th]
                            mm_burst(ps1, wsb[0], th, range(HD, DT))
                            mm_burst(ps3, wsb[1], th, range(HD, DT))
                            epilogue(ps1, ps3, g, htl, th)
                    first_htl = 2
                else:
                    first_htl = 0
                for htl in range(first_htl, HTQ):
                    ht = hq * HTQ + htl
                    w1sb, w3sb = w_pre if htl == 0 else load_w(ht)
                    for th in range(TH):
                        ps1 = pspool.tile([P, NT], F32, tag="ps", bufs=4, name="ps1")
                        ps3 = pspool.tile([P, NT], F32, tag="ps", bufs=4, name="ps3")
                        mm_burst(ps1, w1sb, th, range(DT))
                        mm_burst(ps3, w3sb, th, range(DT))
                        epilogue(ps1, ps3, g, htl, th)

                # prefetch next phase's first weight tiles ahead of the w2
                # stream so phase1 of hq+1 starts without a DMA bubble
                if hq + 1 < HQ:
                    w_pre = load_w((hq + 1) * HTQ)

                for dtt in range(DTT):
                    w2sb = w2pool.tile([P, HTQ, P], F16, tag="w2")
                    nc.sync.dma_start(w2sb, w2p[hq, dtt])
                    # one double-bank psum for both t-halves: half the
                    # accumulation groups and DVE ops in phase 2
                    po = pspool.tile([P, T], F32, tag="po2", bufs=2, name="po")
                    for th in range(TH):
                        ts = slice(th * NT, (th + 1) * NT)
                        for htl in range(HTQ):
                            nc.tensor.matmul(
                                po[:, ts],
                                lhsT=w2sb[:, htl],
                                rhs=g[:, htl, ts],
                                start=(htl == 0),
                                stop=(htl == HTQ - 1),
                            )
                    if hq == 0:
                        nc.vector.tensor_copy(out=out_acc[:, dtt], in_=po)
                    elif hq < HQ - 1:
                        nc.vector.tensor_add(
                            out=out_acc[:, dtt],
                            in0=out_acc[:, dtt],
                            in1=po,
                        )
                    else:
                        # final hq: per-t-half add + store so the tail after
                        # the very last matmul is one [128,512] add + DMA
                        for th in range(TH):
                            ts = slice(th * NT, (th + 1) * NT)
                            nc.vector.tensor_add(
                                out=out_acc[:, dtt, ts],
                                in0=out_acc[:, dtt, ts],
                                in1=po[:, ts],
                            )
                            nc.sync.dma_start(
                                outT[dtt * P : (dtt + 1) * P, ts],
                                out_acc[:, dtt, ts],
                            )
    nc.compile()
    return nc


def _pack_inputs(x, w1, w2, w3):
    """Per-expert host-side packing into DMA-linear fp16 layouts."""
    in_maps = []
    for e in range(E):
        xe = np.asarray(x[e], dtype=np.float16)
        w1e = np.asarray(w1[e], dtype=np.float16)
        w2e = np.asarray(w2[e], dtype=np.float16)
        w3e = np.asarray(w3[e], dtype=np.float16)
        # xp[dt, p, t] = x[t, dt*128+p]
        xp = np.ascontiguousarray(xe.reshape(T, DT, P).transpose(1, 2, 0))
        # w1p[ht, p, dt, h] = w1[dt*128+p, ht*128+h]
        w1p = np.ascontiguousarray(
            w1e.reshape(DT, P, HT, P).transpose(2, 1, 0, 3)
        )
        # w3p[ht, p, dt, h] = w3[ht*128+h, dt*128+p]
        w3p = np.ascontiguousarray(
            w3e.reshape(HT, P, DT, P).transpose(0, 3, 2, 1)
        )
        # w2p[hq, dtt, p, htl, d] = w2[dtt*128+d, (hq*HTQ+htl)*128+p]
        w2p = np.ascontiguousarray(
            w2e.reshape(DTT, P, HQ, HTQ, P).transpose(2, 0, 4, 3, 1)
        )
        in_maps.append({"xp": xp, "w1p": w1p, "w3p": w3p, "w2p": w2p})
    return in_maps


def kernel(x, w1, w2, w3, _trace=False, _trace_kwargs=None):
    if "nc" not in _CACHE:
        _CACHE["nc"] = _build_nc()
    nc = _CACHE["nc"]
    in_maps = _pack_inputs(x, w1, w2, w3)
    kw = {}
    if _trace:
        kw = {"trace": True}
        if _trace_kwargs:
            kw.update(_trace_kwargs)
    res = run_bass_kernel_spmd(nc, in_maps, core_ids=list(range(E)), **kw)
    out = np.empty((E, T, D), dtype=np.float32)
    for e in range(E):
        out[e] = res.results[e]["outT"].T
    if _trace:
        _CACHE["last_results"] = res
    return out


# revision 4
# speedup vs baseline: 1.0711x; 1.0004x over previous
"""Grouped SwiGLU FFN (8 experts) — expert-parallel Bass kernel for 8 trn2 cores.

Per core (one expert): out = (silu(x@w1) * (x@w3T)) @ w2T.
  x: [T=1024, D=2048], w1: [D, H=4096], w3: [H, D], w2: [D, H].

Device-side formulation (matmul operands in fp16 at full PE rate — halves
LDWEIGHTS time vs fp32r so the PE pitch hits the 1-col/cycle streaming
bound — zero on-device transposes; layouts are pre-packed on host):
  phase1: g^T[h, t]  = silu(w1^T-tile.T @ x^T) * (w3-tile.T @ x^T)  per h-tile,
          all 32 h-tiles kept resident in SBUF as fp16 (8 MB)
  phase2: out^T[d,t] = w2-tile.T @ g^T, one 32-matmul PSUM accumulation per
          (d-tile, t-half) — no SBUF accumulator, tail is one copy + DMA

Startup: a few dummy "burn" matmuls on scratch SBUF ramp the PE clock while
the DMA rings initialize; startup DMAs are issued in exact first-use order.
"""

import sys

sys.path.insert(0, "/opt/trn_rl_repo")

import numpy as np

import concourse.bass as bass
from concourse import bacc
import concourse.mybir as mybir
import concourse.tile as tile
from concourse.bass_utils import run_bass_kernel_spmd

E, T, D, H = 8, 1024, 2048, 4096
P = 128
NT = 512            # matmul moving free dim (psum bank limit for fp32 out)
DT = D // P         # 16 contraction tiles over D
HT = H // P         # 32 h-tiles
TH = T // NT        # 2 t-halves
DTT = D // P        # 16 out^T row tiles
HD = DT // 2        # half of the contraction tiles (warm-start split)
NBURN = 8           # clock-ramp matmuls issued before real work
F32 = mybir.dt.float32
F16 = mybir.dt.float16

_CACHE: dict = {}


def _build_nc():
    nc = bacc.Bacc("TRN2", target_bir_lowering=False, debug=False)
    xp = nc.dram_tensor("xp", [DT, P, T], F16, kind="ExternalInput")
    w1p = nc.dram_tensor("w1p", [HT, P, DT, P], F16, kind="ExternalInput")
    w3p = nc.dram_tensor("w3p", [HT, P, DT, P], F16, kind="ExternalInput")
    w2p = nc.dram_tensor("w2p", [DTT, P, HT, P], F16, kind="ExternalInput")
    outT = nc.dram_tensor("outT", [D, T], F32, kind="ExternalOutput")

    with tile.TileContext(nc) as tc:
        with (
            tc.tile_pool(name="xpool", bufs=1) as xpool,
            tc.tile_pool(name="gpool", bufs=1) as gpool,
            tc.tile_pool(name="wpool", bufs=3) as wpool,
            tc.tile_pool(name="w2pool", bufs=3) as w2pool,
            tc.tile_pool(name="spool", bufs=1) as spool,
            tc.tile_pool(name="ospool", bufs=4) as ospool,
            tc.tile_pool(name="pspool", bufs=8, space="PSUM") as pspool,
        ):
            # --- PE clock pre-burn on zeroed scratch (no DMA dependency):
            # keeps the Tensor engine busy through the DMA-ring init window
            # so real matmuls start at the full 2.4 GHz p-state
            burnw = spool.tile([P, P], F16, tag="burnw")
            burnx = spool.tile([P, NT], F16, tag="burnx")
            nc.vector.memset(burnw, 0.0)
            nc.vector.memset(burnx, 0.0)
            psb = pspool.tile([P, NT], F32, tag="ps", bufs=4, name="psburn")
            for i in range(NBURN):
                nc.tensor.matmul(
                    psb,
                    lhsT=burnw,
                    rhs=burnx,
                    start=(i == 0),
                    stop=(i == NBURN - 1),
                )

            def load_w(ht):
                w1sb = wpool.tile([P, DT, P], F16, tag="w1", name=f"w1sb_{ht}")
                nc.sync.dma_start(w1sb, w1p[ht])
                w3sb = wpool.tile([P, DT, P], F16, tag="w3", name=f"w3sb_{ht}")
                nc.sync.dma_start(w3sb, w3p[ht])
                return w1sb, w3sb

            # --- startup: interleave DMA issue in exact first-use order so
            # the PE's warm bursts start as soon as possible
            w1sb0 = wpool.tile([P, DT, P], F16, tag="w1", name="w1sb_0")
            w3sb0 = wpool.tile([P, DT, P], F16, tag="w3", name="w3sb_0")
            w1sb1 = wpool.tile([P, DT, P], F16, tag="w1", name="w1sb_1")
            w3sb1 = wpool.tile([P, DT, P], F16, tag="w3", name="w3sb_1")
            xsb = xpool.tile([P, DT, T], F16, tag="x")

            def wdma(sb, src, half):
                sl = slice(half * HD, (half + 1) * HD)
                nc.sync.dma_start(sb[:, sl], src[:, sl])

            def xdma(dt_i):
                # full-T chunk: 2KB contiguous per partition line
                nc.sync.dma_start(xsb[:, dt_i], xp[dt_i])

            wdma(w1sb0, w1p[0], 0)
            for dt_i in range(HD):
                xdma(dt_i)
            wdma(w3sb0, w3p[0], 0)
            wdma(w1sb1, w1p[1], 0)
            wdma(w3sb1, w3p[1], 0)
            wdma(w1sb0, w1p[0], 1)
            wdma(w3sb0, w3p[0], 1)
            for dt_i in range(HD, DT):
                xdma(dt_i)
            wdma(w1sb1, w1p[1], 1)
            wdma(w3sb1, w3p[1], 1)

            g = gpool.tile([P, HT, T], F16, tag="g")

            def mm_burst(ps, wsb, th, dts):
                ts = slice(th * NT, (th + 1) * NT)
                for dt_i in dts:
                    nc.tensor.matmul(
                        ps,
                        lhsT=wsb[:, dt_i],
                        rhs=xsb[:, dt_i, ts],
                        start=(dt_i == 0),
                        stop=(dt_i == DT - 1),
                    )

            def epilogue(ps1, ps3, ht, th):
                ts = slice(th * NT, (th + 1) * NT)
                sil = spool.tile([P, NT], F32, tag="sil")
                nc.scalar.activation(
                    sil, ps1, mybir.ActivationFunctionType.Silu
                )
                nc.vector.tensor_mul(out=g[:, ht, ts], in0=sil, in1=ps3)

            # --- phase 1: warm start splits the first two h-tiles'
            # accumulation into dt halves so 8 psum groups are in the PE
            # queue while the x chunks stream in
            warm = {}
            for ht in range(2):
                wsb = (w1sb0, w3sb0) if ht == 0 else (w1sb1, w3sb1)
                for th in range(TH):
                    ps1 = pspool.tile([P, NT], F32, tag="ps", bufs=4, name="ps1")
                    ps3 = pspool.tile([P, NT], F32, tag="ps", bufs=4, name="ps3")
                    warm[ht, th] = (ps1, ps3, wsb)
                    mm_burst(ps1, wsb[0], th, range(HD))
                    mm_burst(ps3, wsb[1], th, range(HD))
            for ht in range(2):
                for th in range(TH):
                    ps1, ps3, wsb = warm[ht, th]
                    mm_burst(ps1, wsb[0], th, range(HD, DT))
                    mm_burst(ps3, wsb[1], th, range(HD, DT))
                    epilogue(ps1, ps3, ht, th)

            for ht in range(2, HT):
                w1sb, w3sb = load_w(ht)
                for th in range(TH):
                    ps1 = pspool.tile([P, NT], F32, tag="ps", bufs=4, name="ps1")
                    ps3 = pspool.tile([P, NT], F32, tag="ps", bufs=4, name="ps3")
                    mm_burst(ps1, w1sb, th, range(DT))
                    mm_burst(ps3, w3sb, th, range(DT))
                    epilogue(ps1, ps3, ht, th)

            # --- phase 2: per (d-tile, t-half), one 32-matmul accumulation
            # over the whole H in a single psum bank, then copy + store
            for dtt in range(DTT):
                w2sb = w2pool.tile([P, HT, P], F16, tag="w2")
                nc.sync.dma_start(w2sb, w2p[dtt])
                for th in range(TH):
                    ts = slice(th * NT, (th + 1) * NT)
                    po = pspool.tile([P, NT], F32, tag="po", bufs=4, name="po")
                    for ht in range(HT):
                        nc.tensor.matmul(
                            po,
                            lhsT=w2sb[:, ht],
                            rhs=g[:, ht, ts],
                            start=(ht == 0),
                            stop=(ht == HT - 1),
                        )
                    osb = ospool.tile([P, NT], F32, tag="osb")
                    nc.vector.tensor_copy(out=osb, in_=po)
                    nc.sync.dma_start(
                        outT[dtt * P : (dtt + 1) * P, ts], osb
                    )
    nc.compile()
    return nc


def _pack_inputs(x, w1, w2, w3):
    """Per-expert host-side packing into DMA-linear fp16 layouts."""
    in_maps = []
    for e in range(E):
        xe = np.asarray(x[e], dtype=np.float16)
        w1e = np.asarray(w1[e], dtype=np.float16)
        w2e = np.asarray(w2[e], dtype=np.float16)
        w3e = np.asarray(w3[e], dtype=np.float16)
        # xp[dt, p, t] = x[t, dt*128+p]
        xp = np.ascontiguousarray(xe.reshape(T, DT, P).transpose(1, 2, 0))
        # w1p[ht, p, dt, h] = w1[dt*128+p, ht*128+h]
        w1p = np.ascontiguousarray(
            w1e.reshape(DT, P, HT, P).transpose(2, 1, 0, 3)
        )
        # w3p[ht, p, dt, h] = w3[ht*128+h, dt*128+p]
        w3p = np.ascontiguousarray(
            w3e.reshape(HT, P, DT, P).transpose(0, 3, 2, 1)
        )
        # w2p[dtt, p, ht, d] = w2[dtt*128+d, ht*128+p]
        w2p = np.ascontiguousarray(
            w2e.reshape(DTT, P, HT, P).transpose(0, 3, 2, 1)
        )
        in_maps.append({"xp": xp, "w1p": w1p, "w3p": w3p, "w2p": w2p})
    return in_maps


def kernel(x, w1, w2, w3, _trace=False, _trace_kwargs=None):
    if "nc" not in _CACHE:
        _CACHE["nc"] = _build_nc()
    nc = _CACHE["nc"]
    in_maps = _pack_inputs(x, w1, w2, w3)
    kw = {}
    if _trace:
        kw = {"trace": True}
        if _trace_kwargs:
            kw.update(_trace_kwargs)
    res = run_bass_kernel_spmd(nc, in_maps, core_ids=list(range(E)), **kw)
    out = np.empty((E, T, D), dtype=np.float32)
    for e in range(E):
        out[e] = res.results[e]["outT"].T
    if _trace:
        _CACHE["last_results"] = res
    return out


# revision 7
# speedup vs baseline: 1.0773x; 1.0058x over previous
"""Grouped SwiGLU FFN (8 experts) — expert-parallel Bass kernel for 8 trn2 cores.

Per core (one expert): out = (silu(x@w1) * (x@w3T)) @ w2T.
  x: [T=1024, D=2048], w1: [D, H=4096], w3: [H, D], w2: [D, H].

Device-side formulation (matmul operands in fp16 at full PE rate — halves
LDWEIGHTS time vs fp32r so the PE pitch hits the 1-col/cycle streaming
bound — zero on-device transposes; layouts are pre-packed on host):
  phase1: g^T[h, t]  = silu(w1^T-tile.T @ x^T) * (w3-tile.T @ x^T)  per h-tile,
          all 32 h-tiles kept resident in SBUF as fp16 (8 MB)
  phase2: out^T[d,t] = w2-tile.T @ g^T, one 32-matmul PSUM accumulation per
          (d-tile, t-half) — no SBUF accumulator, tail is one copy + DMA

Startup: tiny "burn" matmuls on scratch SBUF ramp the PE clock while the
DMA rings initialize; startup DMAs are issued in exact first-use order and
the first two h-tiles' matmuls are interleaved across 8 accumulation groups
(all 8 psum banks) so each arriving x chunk feeds 8 matmuls — chunk-paced,
gap-free.
"""

import sys

sys.path.insert(0, "/opt/trn_rl_repo")

import numpy as np

import concourse.bass as bass
from concourse import bacc
import concourse.mybir as mybir
import concourse.tile as tile
from concourse.bass_utils import run_bass_kernel_spmd

E, T, D, H = 8, 1024, 2048, 4096
P = 128
NT = 512            # matmul moving free dim (ISA limit)
DT = D // P         # 16 contraction tiles over D
HT = H // P         # 32 h-tiles
TH = T // NT        # 2 t-halves
DTT = D // P        # 16 out^T row tiles
HD = DT // 2        # half of the contraction tiles
NBURN = 56          # tiny clock-ramp matmuls issued before real work
NTB = 64            # burn matmul moving size
F32 = mybir.dt.float32
F16 = mybir.dt.float16

_CACHE: dict = {}


def _build_nc():
    nc = bacc.Bacc("TRN2", target_bir_lowering=False, debug=False)
    xp = nc.dram_tensor("xp", [DT, P, T], F16, kind="ExternalInput")
    w1p = nc.dram_tensor("w1p", [HT, P, DT, P], F16, kind="ExternalInput")
    w3p = nc.dram_tensor("w3p", [HT, P, DT, P], F16, kind="ExternalInput")
    w2p = nc.dram_tensor("w2p", [DTT, P, HT, P], F16, kind="ExternalInput")
    outT = nc.dram_tensor("outT", [D, T], F32, kind="ExternalOutput")

    with tile.TileContext(nc) as tc:
        with (
            tc.tile_pool(name="xpool", bufs=1) as xpool,
            tc.tile_pool(name="gpool", bufs=1) as gpool,
            tc.tile_pool(name="wpool", bufs=3) as wpool,
            tc.tile_pool(name="w2pool", bufs=3) as w2pool,
            tc.tile_pool(name="spool", bufs=1) as spool,
            tc.tile_pool(name="ospool", bufs=4) as ospool,
            tc.tile_pool(name="pspool", bufs=8, space="PSUM") as pspool,
        ):
            # --- PE clock pre-burn on zeroed scratch (no DMA dependency):
            # keeps the Tensor engine busy through the DMA-ring init window
            # so real matmuls start at the full 2.4 GHz p-state. Tiny moving
            # dim so the burn quantizes finely and overshoot is negligible.
            burnw = spool.tile([P, P], F16, tag="burnw")
            burnx = spool.tile([P, NTB], F16, tag="burnx")
            nc.vector.memset(burnw, 0.0)
            nc.vector.memset(burnx, 0.0)
            psb = pspool.tile([P, NT], F32, tag="po", bufs=4, name="psburn")
            for i in range(NBURN):
                nc.tensor.matmul(
                    psb[:, 0:NTB],
                    lhsT=burnw,
                    rhs=burnx,
                    start=(i == 0),
                    stop=(i == NBURN - 1),
                )

            def load_w(ht):
                w1sb = wpool.tile([P, DT, P], F16, tag="w1", name=f"w1sb_{ht}")
                nc.sync.dma_start(w1sb, w1p[ht])
                w3sb = wpool.tile([P, DT, P], F16, tag="w3", name=f"w3sb_{ht}")
                nc.sync.dma_start(w3sb, w3p[ht])
                return w1sb, w3sb

            # --- startup: interleave DMA issue in exact first-use order so
            # the PE's warm matmuls start as soon as possible
            w1sb0 = wpool.tile([P, DT, P], F16, tag="w1", name="w1sb_0")
            w3sb0 = wpool.tile([P, DT, P], F16, tag="w3", name="w3sb_0")
            w1sb1 = wpool.tile([P, DT, P], F16, tag="w1", name="w1sb_1")
            w3sb1 = wpool.tile([P, DT, P], F16, tag="w3", name="w3sb_1")
            xsb = xpool.tile([P, DT, T], F16, tag="x")

            def wdma(sb, src, half):
                sl = slice(half * HD, (half + 1) * HD)
                nc.sync.dma_start(sb[:, sl], src[:, sl])

            def xdma(dt_i):
                # full-T chunk: 2KB contiguous per partition line
                nc.sync.dma_start(xsb[:, dt_i], xp[dt_i])

            wdma(w1sb0, w1p[0], 0)
            xdma(0)
            wdma(w3sb0, w3p[0], 0)
            wdma(w1sb1, w1p[1], 0)
            wdma(w3sb1, w3p[1], 0)
            for dt_i in range(1, HD):
                xdma(dt_i)
            wdma(w1sb0, w1p[0], 1)
            wdma(w3sb0, w3p[0], 1)
            wdma(w1sb1, w1p[1], 1)
            wdma(w3sb1, w3p[1], 1)
            for dt_i in range(HD, DT):
                xdma(dt_i)

            g = gpool.tile([P, HT, T], F16, tag="g")

            def epilogue(ps1, ps3, ht, th):
                ts = slice(th * NT, (th + 1) * NT)
                sil = spool.tile([P, NT], F32, tag="sil")
                nc.scalar.activation(
                    sil, ps1, mybir.ActivationFunctionType.Silu
                )
                nc.vector.tensor_mul(out=g[:, ht, ts], in0=sil, in1=ps3)

            # --- phase 1 warm start: first two h-tiles interleaved across
            # 8 accumulation groups (all 8 psum banks) so each arriving x
            # chunk feeds 8 matmuls — the PE stays chunk-paced, gap-free
            wgrp = []
            for ht in range(2):
                # ht0 pairs on the "ps" banks, ht1 pairs on the "po" banks
                tag = "ps" if ht == 0 else "po"
                wsb = (w1sb0, w3sb0) if ht == 0 else (w1sb1, w3sb1)
                for th in range(TH):
                    ps1 = pspool.tile([P, NT], F32, tag=tag, bufs=4, name="ps1")
                    ps3 = pspool.tile([P, NT], F32, tag=tag, bufs=4, name="ps3")
                    wgrp.append((ps1, wsb[0], th, ht))
                    wgrp.append((ps3, wsb[1], th, ht))
            for dt_i in range(DT):
                for ps, wsb, th, _ht in wgrp:
                    ts = slice(th * NT, (th + 1) * NT)
                    nc.tensor.matmul(
                        ps,
                        lhsT=wsb[:, dt_i],
                        rhs=xsb[:, dt_i, ts],
                        start=(dt_i == 0),
                        stop=(dt_i == DT - 1),
                    )
            for i in range(0, 8, 2):
                ps1, _, th, ht = wgrp[i]
                ps3 = wgrp[i + 1][0]
                epilogue(ps1, ps3, ht, th)

            for ht in range(2, HT):
                w1sb, w3sb = load_w(ht)
                for th in range(TH):
                    ps1 = pspool.tile([P, NT], F32, tag="ps", bufs=4, name="ps1")
                    ps3 = pspool.tile([P, NT], F32, tag="ps", bufs=4, name="ps3")
                    ts = slice(th * NT, (th + 1) * NT)
                    for dt_i in range(DT):
                        nc.tensor.matmul(
                            ps1,
                            lhsT=w1sb[:, dt_i],
                            rhs=xsb[:, dt_i, ts],
                            start=(dt_i == 0),
                            stop=(dt_i == DT - 1),
                        )
                    for dt_i in range(DT):
                        nc.tensor.matmul(
                            ps3,
                            lhsT=w3sb[:, dt_i],
                            rhs=xsb[:, dt_i, ts],
                            start=(dt_i == 0),
                            stop=(dt_i == DT - 1),
                        )
                    epilogue(ps1, ps3, ht, th)

            # --- phase 2: per (d-tile, t-half), one 32-matmul accumulation
            # over the whole H in a single psum bank, then copy + store
            for dtt in range(DTT):
                w2sb = w2pool.tile([P, HT, P], F16, tag="w2")
                nc.sync.dma_start(w2sb, w2p[dtt])
                for th in range(TH):
                    ts = slice(th * NT, (th + 1) * NT)
                    po = pspool.tile([P, NT], F32, tag="po", bufs=4, name="po")
                    for ht in range(HT):
                        nc.tensor.matmul(
                            po,
                            lhsT=w2sb[:, ht],
                            rhs=g[:, ht, ts],
                            start=(ht == 0),
                            stop=(ht == HT - 1),
                        )
                    osb = ospool.tile([P, NT], F32, tag="osb")
                    nc.vector.tensor_copy(out=osb, in_=po)
                    nc.sync.dma_start(
                        outT[dtt * P : (dtt + 1) * P, ts], osb
                    )
    nc.compile()
    return nc


def _pack_inputs(x, w1, w2, w3):
    """Per-expert host-side packing into DMA-linear fp16 layouts."""
    in_maps = []
    for e in range(E):
        xe = np.asarray(x[e], dtype=np.float16)
        w1e = np.asarray(w1[e], dtype=np.float16)
        w2e = np.asarray(w2[e], dtype=np.float16)
        w3e = np.asarray(w3[e], dtype=np.float16)
        # xp[dt, p, t] = x[t, dt*128+p]
        xp = np.ascontiguousarray(xe.reshape(T, DT, P).transpose(1, 2, 0))
        # w1p[ht, p, dt, h] = w1[dt*128+p, ht*128+h]
        w1p = np.ascontiguousarray(
            w1e.reshape(DT, P, HT, P).transpose(2, 1, 0, 3)
        )
        # w3p[ht, p, dt, h] = w3[ht*128+h, dt*128+p]
        w3p = np.ascontiguousarray(
            w3e.reshape(HT, P, DT, P).transpose(0, 3, 2, 1)
        )
        # w2p[dtt, p, ht, d] = w2[dtt*128+d, ht*128+p]
        w2p = np.ascontiguousarray(
            w2e.reshape(DTT, P, HT, P).transpose(0, 3, 2, 1)
        )
        in_maps.append({"xp": xp, "w1p": w1p, "w3p": w3p, "w2p": w2p})
    return in_maps


def kernel(x, w1, w2, w3, _trace=False, _trace_kwargs=None):
    if "nc" not in _CACHE:
        _CACHE["nc"] = _build_nc()
    nc = _CACHE["nc"]
    in_maps = _pack_inputs(x, w1, w2, w3)
    kw = {}
    if _trace:
        kw = {"trace": True}
        if _trace_kwargs:
            kw.update(_trace_kwargs)
    res = run_bass_kernel_spmd(nc, in_maps, core_ids=list(range(E)), **kw)
    out = np.empty((E, T, D), dtype=np.float32)
    for e in range(E):
        out[e] = res.results[e]["outT"].T
    if _trace:
        _CACHE["last_results"] = res
    return out


# revision 14
# speedup vs baseline: 1.0780x; 1.0007x over previous
"""Grouped SwiGLU FFN (8 experts) — expert-parallel Bass kernel for 8 trn2 cores.

Per core (one expert): out = (silu(x@w1) * (x@w3T)) @ w2T.
  x: [T=1024, D=2048], w1: [D, H=4096], w3: [H, D], w2: [D, H].

Device-side formulation (matmul operands in fp16 at full PE rate — halves
LDWEIGHTS time vs fp32r so the PE pitch hits the 1-col/cycle streaming
bound — zero on-device transposes; layouts are pre-packed on host):
  phase1: g^T[h, t]  = silu(w1^T-tile.T @ x^T) * (w3-tile.T @ x^T)  per h-tile,
          all 32 h-tiles kept resident in SBUF as fp16 (8 MB)
  phase2: out^T[d,t] = w2-tile.T @ g^T, one 32-matmul PSUM accumulation per
          (d-tile, t-half) — no SBUF accumulator, tail is one copy + DMA

Startup: tiny "burn" matmuls on scratch SBUF ramp the PE clock while the
DMA rings initialize; startup DMAs are issued in exact first-use order and
the first two h-tiles' matmuls are interleaved across 8 accumulation groups
(all 8 psum banks) so each arriving x chunk feeds 8 matmuls — chunk-paced,
gap-free.
"""

import sys

sys.path.insert(0, "/opt/trn_rl_repo")

import numpy as np

import concourse.bass as bass
from concourse import bacc
import concourse.mybir as mybir
import concourse.tile as tile
from concourse.bass_utils import run_bass_kernel_spmd

E, T, D, H = 8, 1024, 2048, 4096
P = 128
NT = 512            # matmul moving free dim (ISA limit)
DT = D // P         # 16 contraction tiles over D
HT = H // P         # 32 h-tiles
TH = T // NT        # 2 t-halves
DTT = D // P        # 16 out^T row tiles
HD = DT // 2        # half of the contraction tiles
NBURN = 62          # tiny clock-ramp matmuls issued before real work
NTB = 64            # burn matmul moving size
F32 = mybir.dt.float32
F16 = mybir.dt.float16

_CACHE: dict = {}


def _build_nc():
    nc = bacc.Bacc("TRN2", target_bir_lowering=False, debug=False)
    xp = nc.dram_tensor("xp", [DT, P, T], F16, kind="ExternalInput")
    w1p = nc.dram_tensor("w1p", [HT, P, DT, P], F16, kind="ExternalInput")
    w3p = nc.dram_tensor("w3p", [HT, P, DT, P], F16, kind="ExternalInput")
    w2p = nc.dram_tensor("w2p", [DTT, P, HT, P], F16, kind="ExternalInput")
    outT = nc.dram_tensor("outT", [D, T], F16, kind="ExternalOutput")

    with tile.TileContext(nc) as tc:
        with (
            tc.tile_pool(name="xpool", bufs=1) as xpool,
            tc.tile_pool(name="gpool", bufs=1) as gpool,
            tc.tile_pool(name="wpool", bufs=3) as wpool,
            tc.tile_pool(name="w2pool", bufs=3) as w2pool,
            tc.tile_pool(name="spool", bufs=1) as spool,
            tc.tile_pool(name="ospool", bufs=4) as ospool,
            tc.tile_pool(name="pspool", bufs=8, space="PSUM") as pspool,
        ):
            # --- PE clock pre-burn on zeroed scratch (no DMA dependency):
            # keeps the Tensor engine busy through the DMA-ring init window
            # so real matmuls start at the full 2.4 GHz p-state. Tiny moving
            # dim so the burn quantizes finely and overshoot is negligible.
            burnw = spool.tile([P, P], F16, tag="burnw")
            burnx = spool.tile([P, NTB], F16, tag="burnx")
            nc.vector.memset(burnw, 0.0)
            nc.vector.memset(burnx, 0.0)
            psb = pspool.tile([P, NT], F32, tag="po", bufs=4, name="psburn")
            for i in range(NBURN):
                nc.tensor.matmul(
                    psb[:, 0:NTB],
                    lhsT=burnw,
                    rhs=burnx,
                    start=(i == 0),
                    stop=(i == NBURN - 1),
                )

            def load_w(ht):
                w1sb = wpool.tile([P, DT, P], F16, tag="w1", name=f"w1sb_{ht}")
                nc.sync.dma_start(w1sb, w1p[ht])
                w3sb = wpool.tile([P, DT, P], F16, tag="w3", name=f"w3sb_{ht}")
                nc.sync.dma_start(w3sb, w3p[ht])
                return w1sb, w3sb

            # --- startup: interleave DMA issue in exact first-use order so
            # the PE's warm matmuls start as soon as possible
            w1sb0 = wpool.tile([P, DT, P], F16, tag="w1", name="w1sb_0")
            w3sb0 = wpool.tile([P, DT, P], F16, tag="w3", name="w3sb_0")
            w1sb1 = wpool.tile([P, DT, P], F16, tag="w1", name="w1sb_1")
            w3sb1 = wpool.tile([P, DT, P], F16, tag="w3", name="w3sb_1")
            xsb = xpool.tile([P, DT, T], F16, tag="x")

            def wdma(sb, src, half):
                sl = slice(half * HD, (half + 1) * HD)
                nc.sync.dma_start(sb[:, sl], src[:, sl])

            def xdma(dt_i):
                # full-T chunk: 2KB contiguous per partition line
                nc.sync.dma_start(xsb[:, dt_i], xp[dt_i])

            wdma(w1sb0, w1p[0], 0)
            # first chunk in t-halves: matmul 1 needs only the th0 slice
            nc.sync.dma_start(xsb[:, 0, 0:NT], xp[0, :, 0:NT])
            nc.sync.dma_start(xsb[:, 0, NT:T], xp[0, :, NT:T])
            wdma(w3sb0, w3p[0], 0)
            wdma(w1sb1, w1p[1], 0)
            wdma(w3sb1, w3p[1], 0)
            for dt_i in range(1, HD):
                xdma(dt_i)
            wdma(w1sb0, w1p[0], 1)
            wdma(w3sb0, w3p[0], 1)
            wdma(w1sb1, w1p[1], 1)
            wdma(w3sb1, w3p[1], 1)
            for dt_i in range(HD, DT):
                xdma(dt_i)

            g = gpool.tile([P, HT, T], F16, tag="g")

            def epilogue(ps1, ps3, ht, th):
                ts = slice(th * NT, (th + 1) * NT)
                sil = spool.tile([P, NT], F32, tag="sil")
                nc.scalar.activation(
                    sil, ps1, mybir.ActivationFunctionType.Silu
                )
                nc.vector.tensor_mul(out=g[:, ht, ts], in0=sil, in1=ps3)

            # --- phase 1 warm start: first two h-tiles interleaved across
            # 8 accumulation groups (all 8 psum banks) so each arriving x
            # chunk feeds 8 matmuls — the PE stays chunk-paced, gap-free
            wgrp = []
            for ht in range(2):
                # ht0 pairs on the "ps" banks, ht1 pairs on the "po" banks
                tag = "ps" if ht == 0 else "po"
                wsb = (w1sb0, w3sb0) if ht == 0 else (w1sb1, w3sb1)
                pairs = []
                for th in range(TH):
                    ps1 = pspool.tile([P, NT], F32, tag=tag, bufs=4, name="ps1")
                    ps3 = pspool.tile([P, NT], F32, tag=tag, bufs=4, name="ps3")
                    pairs.append((ps1, ps3))
                # w1 groups before w3 groups: defers the w3 DMA dependency
                # by one more matmul slot at startup
                for th in range(TH):
                    wgrp.append((pairs[th][0], wsb[0], th, ht))
                for th in range(TH):
                    wgrp.append((pairs[th][1], wsb[1], th, ht))
            for dt_i in range(DT):
                for ps, wsb, th, _ht in wgrp:
                    ts = slice(th * NT, (th + 1) * NT)
                    nc.tensor.matmul(
                        ps,
                        lhsT=wsb[:, dt_i],
                        rhs=xsb[:, dt_i, ts],
                        start=(dt_i == 0),
                        stop=(dt_i == DT - 1),
                    )
            for i in (0, 1, 4, 5):
                ps1, _, th, ht = wgrp[i]
                ps3 = wgrp[i + 2][0]
                epilogue(ps1, ps3, ht, th)

            for ht in range(2, HT):
                w1sb, w3sb = load_w(ht)
                for th in range(TH):
                    ps1 = pspool.tile([P, NT], F32, tag="ps", bufs=4, name="ps1")
                    ps3 = pspool.tile([P, NT], F32, tag="ps", bufs=4, name="ps3")
                    ts = slice(th * NT, (th + 1) * NT)
                    for dt_i in range(DT):
                        nc.tensor.matmul(
                            ps1,
                            lhsT=w1sb[:, dt_i],
                            rhs=xsb[:, dt_i, ts],
                            start=(dt_i == 0),
                            stop=(dt_i == DT - 1),
                        )
                    for dt_i in range(DT):
                        nc.tensor.matmul(
                            ps3,
                            lhsT=w3sb[:, dt_i],
                            rhs=xsb[:, dt_i, ts],
                            start=(dt_i == 0),
                            stop=(dt_i == DT - 1),
                        )
                    epilogue(ps1, ps3, ht, th)

            # --- phase 2: per (d-tile, t-half), one 32-matmul accumulation
            # over the whole H in a single psum bank, then copy + store
            for dtt in range(DTT):
                w2sb = w2pool.tile([P, HT, P], F16, tag="w2")
                nc.sync.dma_start(w2sb, w2p[dtt])
                for th in range(TH):
                    ts = slice(th * NT, (th + 1) * NT)
                    po = pspool.tile([P, NT], F32, tag="po", bufs=4, name="po")
                    for ht in range(HT):
                        nc.tensor.matmul(
                            po,
                            lhsT=w2sb[:, ht],
                            rhs=g[:, ht, ts],
                            start=(ht == 0),
                            stop=(ht == HT - 1),
                        )
                    osb = ospool.tile([P, NT], F16, tag="osb")
                    nc.vector.tensor_copy(out=osb, in_=po)
                    nc.sync.dma_start(
                        outT[dtt * P : (dtt + 1) * P, ts], osb
                    )
    nc.compile()
    return nc


def _pack_inputs(x, w1, w2, w3):
    """Per-expert host-side packing into DMA-linear fp16 layouts."""
    in_maps = []
    for e in range(E):
        xe = np.asarray(x[e], dtype=np.float16)
        w1e = np.asarray(w1[e], dtype=np.float16)
        w2e = np.asarray(w2[e], dtype=np.float16)
        w3e = np.asarray(w3[e], dtype=np.float16)
        # xp[dt, p, t] = x[t, dt*128+p]
        xp = np.ascontiguousarray(xe.reshape(T, DT, P).transpose(1, 2, 0))
        # w1p[ht, p, dt, h] = w1[dt*128+p, ht*128+h]
        w1p = np.ascontiguousarray(
            w1e.reshape(DT, P, HT, P).transpose(2, 1, 0, 3)
        )
        # w3p[ht, p, dt, h] = w3[ht*128+h, dt*128+p]
        w3p = np.ascontiguousarray(
            w3e.reshape(HT, P, DT, P).transpose(0, 3, 2, 1)
        )
        # w2p[dtt, p, ht, d] = w2[dtt*128+d, ht*128+p]
        w2p = np.ascontiguousarray(
            w2e.reshape(DTT, P, HT, P).transpose(0, 3, 2, 1)
        )
        in_maps.append({"xp": xp, "w1p": w1p, "w3p": w3p, "w2p": w2p})
    return in_maps


def kernel(x, w1, w2, w3, _trace=False, _trace_kwargs=None):
    if "nc" not in _CACHE:
        _CACHE["nc"] = _build_nc()
    nc = _CACHE["nc"]
    in_maps = _pack_inputs(x, w1, w2, w3)
    kw = {}
    if _trace:
        kw = {"trace": True}
        if _trace_kwargs:
            kw.update(_trace_kwargs)
    res = run_bass_kernel_spmd(nc, in_maps, core_ids=list(range(E)), **kw)
    out = np.empty((E, T, D), dtype=np.float32)
    for e in range(E):
        out[e] = res.results[e]["outT"].T.astype(np.float32)
    if _trace:
        _CACHE["last_results"] = res
    return out
